# revision 1
# baseline (speedup 1.0000x reference)
"""Trainium2 Bass kernel for nn_Clash_net (clash energy over atom pairs).

Contract: kernel(**inputs) takes FULL (unsharded) numpy inputs as produced by
setup_inputs() and returns the FULL [6] float32 energies output.

Strategy (8 NeuronCores, SPMD):
  - Shard the atom-pairs dimension P across the 8 cores (contiguous split,
    padded with sentinel pairs whose clash contribution is exactly 0).
  - Replicate a packed per-atom table [x, y, z, r] (r = radii[atom_names])
    in DRAM on every core; per-pair endpoint records (16 B) are gathered
    on-device with GPSIMD indirect DMA, one [128,1]-offset call per 128
    records (the only offset form the DGE lowers correctly; measured
    ~0.37 us/call pipelined).
  - Per chunk: compute dist = sqrt(|c0-c1|^2 + eps), base = r0+r1-dist,
    then per class c: acc_c += mask_c * relu(base + tol_c)
    (relu via tensor_scalar add+max, masked sum via tensor_tensor_reduce).
  - Per-core partial [6] sums are returned; the host sums the 8 partials
    and scales by exp(weight[0]) (final unshard step).
"""

import sys

sys.path.insert(0, "/opt/trn_rl_repo")

import numpy as np

import concourse.bass as bass
import concourse.bacc as bacc
import concourse.mybir as mybir
import concourse.tile as tile
from concourse.bass_utils import run_bass_kernel_spmd

F32 = mybir.dt.float32
I32 = mybir.dt.int32
U8 = mybir.dt.uint8

N_CORES = 8
EPS = 1e-12

N_ATOMS = 100000
N_PAIRS = 4000000
N_CLASS = 6

PAIRS_PER_CORE = N_PAIRS // N_CORES  # 500000
CHUNK = 489
N_CHUNKS = 8
COLS = CHUNK * N_CHUNKS  # 3912
P_PAD = 128 * COLS  # 500736 >= 500000


def build_nc(p_pad, chunk, n_chunks, ntab, num_devices=N_CORES, repeat=1):
    """repeat>1 re-runs the whole pair loop (for delta-timing); output scales."""
    assert p_pad == 128 * chunk * n_chunks
    nc = bacc.Bacc(
        "TRN2", target_bir_lowering=False, debug=False, num_devices=num_devices
    )
    idx0 = nc.dram_tensor("idx0", [p_pad], I32, kind="ExternalInput")
    idx1 = nc.dram_tensor("idx1", [p_pad], I32, kind="ExternalInput")
    masks = nc.dram_tensor("masks", [N_CLASS, p_pad], U8, kind="ExternalInput")
    table = nc.dram_tensor("table", [ntab, 4], F32, kind="ExternalInput")
    toll = nc.dram_tensor("toll", [128, N_CLASS], F32, kind="ExternalInput")
    outp = nc.dram_tensor("out", [1, N_CLASS], F32, kind="ExternalOutput")

    with tile.TileContext(nc) as tc:
        with (
            tc.tile_pool(name="const", bufs=1) as cpool,
            tc.tile_pool(name="work", bufs=2) as wpool,
            tc.tile_pool(name="psum", bufs=1, space="PSUM") as ppool,
        ):
            n = chunk
            tolb = cpool.tile([128, N_CLASS], F32)
            nc.sync.dma_start(out=tolb[:], in_=toll[:])
            ones = cpool.tile([128, 1], F32)
            nc.vector.memset(ones[:], 1.0)
            epsb = cpool.tile([128, 1], F32)
            nc.vector.memset(epsb[:], EPS)
            acc = cpool.tile([128, N_CLASS], F32)
            nc.vector.memset(acc[:], 0.0)

            idx0_t = idx0[:].rearrange("(k p q) -> k p q", k=n_chunks, p=128)
            idx1_t = idx1[:].rearrange("(k p q) -> k p q", k=n_chunks, p=128)
            masks_t = masks[:].rearrange("c (k p q) -> c k p q", k=n_chunks, p=128)

            for k in [kk for _ in range(repeat) for kk in range(n_chunks)]:
                i0 = wpool.tile([128, n], I32, tag="i0")
                i1 = wpool.tile([128, n], I32, tag="i1")
                nc.sync.dma_start(out=i0[:], in_=idx0_t[k])
                nc.sync.dma_start(out=i1[:], in_=idx1_t[k])

                g0 = wpool.tile([128, n, 4], F32, tag="g0")
                g1 = wpool.tile([128, n, 4], F32, tag="g1")
                for j in range(n):
                    nc.gpsimd.indirect_dma_start(
                        out=g0[:, j, :],
                        out_offset=None,
                        in_=table[:],
                        in_offset=bass.IndirectOffsetOnAxis(ap=i0[:, j : j + 1], axis=0),
                    )
                for j in range(n):
                    nc.gpsimd.indirect_dma_start(
                        out=g1[:, j, :],
                        out_offset=None,
                        in_=table[:],
                        in_offset=bass.IndirectOffsetOnAxis(ap=i1[:, j : j + 1], axis=0),
                    )

                dx = wpool.tile([128, n], F32, tag="dx")
                dy = wpool.tile([128, n], F32, tag="dy")
                dz = wpool.tile([128, n], F32, tag="dz")
                rs = wpool.tile([128, n], F32, tag="rs")
                nc.vector.tensor_sub(out=dx[:], in0=g0[:, :, 0], in1=g1[:, :, 0])
                nc.vector.tensor_sub(out=dy[:], in0=g0[:, :, 1], in1=g1[:, :, 1])
                nc.vector.tensor_sub(out=dz[:], in0=g0[:, :, 2], in1=g1[:, :, 2])
                nc.vector.tensor_add(out=rs[:], in0=g0[:, :, 3], in1=g1[:, :, 3])

                ss = wpool.tile([128, n], F32, tag="ss")
                t2 = wpool.tile([128, n], F32, tag="t2")
                nc.vector.tensor_mul(out=ss[:], in0=dx[:], in1=dx[:])
                nc.vector.tensor_mul(out=t2[:], in0=dy[:], in1=dy[:])
                nc.vector.tensor_add(out=ss[:], in0=ss[:], in1=t2[:])
                nc.vector.tensor_mul(out=t2[:], in0=dz[:], in1=dz[:])
                nc.vector.tensor_add(out=ss[:], in0=ss[:], in1=t2[:])

                dist = wpool.tile([128, n], F32, tag="dist")
                nc.scalar.activation(
                    out=dist[:],
                    in_=ss[:],
                    func=mybir.ActivationFunctionType.Sqrt,
                    bias=epsb[:],
                )
                base = wpool.tile([128, n], F32, tag="base")
                nc.vector.tensor_sub(out=base[:], in0=rs[:], in1=dist[:])

                for c in range(N_CLASS):
                    mk = wpool.tile([128, n], U8, tag=f"mk{c}")
                    nc.sync.dma_start(out=mk[:], in_=masks_t[c, k])
                    rc = wpool.tile([128, n], F32, tag="rc")
                    nc.vector.tensor_scalar(
                        out=rc[:],
                        in0=base[:],
                        scalar1=tolb[:, c : c + 1],
                        scalar2=0.0,
                        op0=mybir.AluOpType.add,
                        op1=mybir.AluOpType.max,
                    )
                    scr = wpool.tile([128, n], F32, tag="scr")
                    nc.vector.tensor_tensor(
                        out=scr[:], in0=rc[:], in1=mk[:], op=mybir.AluOpType.mult
                    )
                    red = wpool.tile([128, 1], F32, tag="red")
                    nc.vector.tensor_reduce(
                        out=red[:],
                        in_=scr[:],
                        axis=mybir.AxisListType.X,
                        op=mybir.AluOpType.add,
                    )
                    nc.vector.tensor_add(
                        out=acc[:, c : c + 1], in0=acc[:, c : c + 1], in1=red[:]
                    )

            psum = ppool.tile([1, N_CLASS], F32, space="PSUM")
            nc.tensor.matmul(
                out=psum[:], lhsT=ones[:], rhs=acc[:], start=True, stop=True
            )
            out6 = cpool.tile([1, N_CLASS], F32)
            nc.vector.tensor_copy(out=out6[:], in_=psum[:])
            nc.sync.dma_start(out=outp[:], in_=out6[:])

    nc.compile()
    return nc


_NC_CACHE = {}


def _get_nc():
    key = (P_PAD, CHUNK, N_CHUNKS)
    if key not in _NC_CACHE:
        _NC_CACHE[key] = build_nc(P_PAD, CHUNK, N_CHUNKS, N_ATOMS + 2)
    return _NC_CACHE[key]


def _prep_inputs(coords, radii, tollerances, weight, atom_names, atom_pairs, clash_masks):
    """Host-side shard/layout prep. Returns (in_maps, exp_weight)."""
    coords = np.asarray(coords, dtype=np.float32)
    radii = np.asarray(radii, dtype=np.float32)
    tollerances = np.asarray(tollerances, dtype=np.float32)
    atom_names = np.asarray(atom_names)
    atom_pairs = np.asarray(atom_pairs)
    clash_masks = np.asarray(clash_masks)

    ntab = N_ATOMS + 2
    table = np.empty((ntab, 4), dtype=np.float32)
    table[:N_ATOMS, :3] = coords
    table[:N_ATOMS, 3] = radii[atom_names.astype(np.int64)]
    table[N_ATOMS] = (1e6, 1e6, 1e6, 0.0)
    table[N_ATOMS + 1] = (-1e6, -1e6, -1e6, 0.0)

    pairs32 = np.ascontiguousarray(atom_pairs.astype(np.int32))
    masks8 = np.ascontiguousarray(clash_masks).view(np.uint8)
    toll2d = np.ascontiguousarray(
        np.broadcast_to(tollerances.reshape(1, N_CLASS), (128, N_CLASS))
    )

    in_maps = []
    for c in range(N_CORES):
        lo, hi = c * PAIRS_PER_CORE, (c + 1) * PAIRS_PER_CORE
        i0 = np.full(P_PAD, N_ATOMS, dtype=np.int32)
        i1 = np.full(P_PAD, N_ATOMS + 1, dtype=np.int32)
        i0[:PAIRS_PER_CORE] = pairs32[lo:hi, 0]
        i1[:PAIRS_PER_CORE] = pairs32[lo:hi, 1]
        m = np.zeros((N_CLASS, P_PAD), dtype=np.uint8)
        m[:, :PAIRS_PER_CORE] = masks8[:, lo:hi]
        in_maps.append(
            {"idx0": i0, "idx1": i1, "masks": m, "table": table, "toll": toll2d}
        )
    return in_maps, float(np.exp(np.float64(np.asarray(weight).reshape(-1)[0])))


def kernel(coords, radii, tollerances, weight, atom_names, atom_pairs, clash_masks):
    nc = _get_nc()
    in_maps, wscale = _prep_inputs(
        coords, radii, tollerances, weight, atom_names, atom_pairs, clash_masks
    )
    res = run_bass_kernel_spmd(nc, in_maps, core_ids=list(range(N_CORES)))
    total = np.zeros(N_CLASS, dtype=np.float64)
    for c in range(N_CORES):
        total += res.results[c]["out"].reshape(N_CLASS).astype(np.float64)
    return (total * wscale).astype(np.float32)

